# revision 41
# baseline (speedup 1.0000x reference)
"""Multi-head attention (B=2, S=2048, D=1024, H=16, Dh=64) on 8 trn2 cores.

Sharding: core c handles batch b = c//4 and head-group g = c%4 (4 heads).

Per-core pipeline (all-bf16 matmul operands, f32 psum):
  - q/k projections in transposed layout [dh, S], v in natural layout
    [S, dh] with an appended ones-column (softmax-denominator trick),
  - simT = k^T q (keys on partitions), 2 heads per kt packed via
    tile_position halves; exp on ScalarE (scale folded in; no
    max-subtraction: scores are ~N(0,1)); mask multiply split between
    DVE (odd kt) and GPSIMD (even kt) in bf16,
  - PV transposed: lhsT = e-tile [keys, q128], rhs = v_aug [keys, 65]
    accumulated over the 16 key tiles -> psum [q, 65] whose col 64 is
    the softmax denominator.  Streams 65 rows/matmul instead of 512,
    halving the PE time of the natural PV layout,
  - normalization: reciprocal_approx_fast + per-partition tensor_scalar
    multiply (the denominator is per-q = per-partition here),
  - one PE transpose per q-subtile (both heads fused into a [128,128]
    identity matmul) back to [dh, q] for the out-projection,
  - all psum->sbuf drains on DVE (GPSIMD cannot touch PSUM).
Scheduling: projections are emitted as <=1.7us chunks interleaved into
the attention block pipeline (FILL/PVT_POP/PROJ_POP tables) so the
Scalar engine's exp stream - the second-busiest engine - is fed from
~6us onward; input DMAs are split across the SP and Pool queues.
Host sums the 4 head-group partials per batch and adds bo.
"""

import os
import sys

for _p in ("/opt/trn_rl_repo", "/root/.axon_site/_ro/trn_rl_repo"):
    if os.path.isdir(_p) and _p not in sys.path:
        sys.path.append(_p)

from contextlib import ExitStack

import ml_dtypes
import numpy as np

import concourse.bass as bass
import concourse.tile as tile
from concourse import bacc
from concourse import mybir

F32 = mybir.dt.float32
BF16 = mybir.dt.bfloat16
U16 = mybir.dt.uint16
U8 = mybir.dt.uint8
FP8 = mybir.dt.float8e4
DR = mybir.MatmulPerfMode.DoubleRow
AF = mybir.ActivationFunctionType
MUL = mybir.AluOpType.mult
AND = mybir.AluOpType.bitwise_and
WARMUP_N = 70  # PE ramp warmup matmuls while input DMAs stream


def build_attention_nc(S=2048, D=1024, HL=4, DH=64, reps=1, upto="full", qk8=False):
    """Bass program for one core: 4 local heads of one batch.

    Inputs : xT [D, S] bf16, maskT [S, S] bf16, Wq/Wk/Wv [D, HL*DH] bf16,
             Wo [HL*DH, D] bf16, ident [128, 128] bf16
    Output : out [S, D] f32 (partial: this head-group's contribution, no bias)
    """
    QB = 512          # q-chunk width
    KB = 128          # key tile (partition dim)
    INNER = HL * DH   # local inner dim (256)
    NP = D // 128     # contraction tiles over D
    NQ = S // QB      # q chunks
    NK = S // KB      # key tiles
    NHP = HL // 2     # head pairs
    # qk8 stores Wq/Wk pre-scaled by 16 (better fp8 range); q@k then carries
    # a 256x factor which the exp scale folds back out
    scale = float(DH) ** -0.5 / (256.0 if qk8 else 1.0)

    assert HL % 2 == 0 and DH == 64 and D % 128 == 0 and S % 512 == 0

    nc = bacc.Bacc(trn_type="TRN2")

    if qk8:
        xT8_d = nc.dram_tensor("xT8", (D, S), FP8, kind="ExternalInput")
        wq8_d = nc.dram_tensor("Wq8", (D, INNER), FP8, kind="ExternalInput")
        wk8_d = nc.dram_tensor("Wk8", (D, INNER), FP8, kind="ExternalInput")
    else:
        xT_d = nc.dram_tensor("xT", (D, S), BF16, kind="ExternalInput")
    maskT_d = nc.dram_tensor("maskT", (S, S), U16, kind="ExternalInput")
    if not qk8:
        wq_d = nc.dram_tensor("Wq", (D, INNER), BF16, kind="ExternalInput")
        wk_d = nc.dram_tensor("Wk", (D, INNER), BF16, kind="ExternalInput")
    wv_d = nc.dram_tensor("Wv", (D, INNER), BF16, kind="ExternalInput")
    wo_d = nc.dram_tensor("Wo", (INNER, D), BF16, kind="ExternalInput")
    ident_d = nc.dram_tensor("ident", (128, 128), BF16, kind="ExternalInput")
    out_d = nc.dram_tensor("out", (S, D), F32, kind="ExternalOutput")

    with tile.TileContext(nc) as tc:
      for rep in range(reps):
       with ExitStack() as ctx:
        persist = ctx.enter_context(tc.tile_pool(name=f"persist{rep}", bufs=1))

        qT = persist.tile([128, NHP, S], BF16)   # [2x64 dh, hp, q]
        kT = persist.tile([128, NHP, S], BF16)
        v_sb = persist.tile([128, NK, HL, DH + 1], BF16)  # v + ones col
        wo_sb = persist.tile([128, NHP, D], BF16)
        o_norm = persist.tile([128, NHP, S], BF16)  # attn out ^T (normalized)
        ident = persist.tile([128, 128], BF16)

        if upto != "full":
            zst = persist.tile([128, D], F32)
            nc.vector.memset(zst[:, :], 0.0)
            nc.sync.dma_start(out=out_d[0:128, :], in_=zst[:, :])
        nc.vector.memset(v_sb[:, :, :, DH : DH + 1], 1.0)  # ones cols only

        simps = ctx.enter_context(
            tc.tile_pool(name=f"simps{rep}", bufs=2, space="PSUM")
        )
        pvtps = ctx.enter_context(
            tc.tile_pool(name=f"pvtps{rep}", bufs=2, space="PSUM")
        )
        auxps = ctx.enter_context(
            tc.tile_pool(name=f"auxps{rep}", bufs=2, space="PSUM")
        )

        ph1 = ctx.enter_context(tc.tile_pool(name=f"ph1_{rep}", bufs=1))
        wv_sb = ph1.tile([128, NP, INNER], BF16)
        if qk8:
            x8 = ph1.tile([128, NP, S], FP8)
            wq_sb = ph1.tile([128, NP, INNER], FP8)
            wk_sb = ph1.tile([128, NP, INNER], FP8)
            xv = x8
        else:
            xts = ph1.tile([128, NP, S], BF16)
            wq_sb = ph1.tile([128, NP, INNER], BF16)
            wk_sb = ph1.tile([128, NP, INNER], BF16)
            xv = xts

        # ident first (tiny), then PE warmup transposes to burn the
        # frequency ramp while the big input DMAs stream in
        nc.sync.dma_start(out=ident[:, :], in_=ident_d[:, :])
        if rep == 0:
            wu = auxps.tile([128, 512], F32, tag="aux")
            for _ in range(WARMUP_N):
                nc.tensor.matmul(
                    wu[0:128, 0:128], lhsT=ident[:, :], rhs=ident[:, :],
                    start=True, stop=True,
                )

        def dma_x(eng, xc):
            src = xT8_d if qk8 else xT_d
            dst = x8 if qk8 else xts
            eng.dma_start(
                out=dst[:, :, xc * QB : (xc + 1) * QB],
                in_=src[:, xc * QB : (xc + 1) * QB].rearrange(
                    "(a p) q -> p a q", p=128
                ),
            )

        mpool = ctx.enter_context(tc.tile_pool(name=f"mp{rep}", bufs=2))
        mb0 = mpool.tile([128, NK, QB], U16, tag="m")

        def m0q(i):  # quarter of block0's mask, priority-interleaved below
            nc.gpsimd.dma_start(
                out=mb0[:, i * 4 : (i + 1) * 4, :],
                in_=maskT_d[i * 512 : (i + 1) * 512, 0:QB].rearrange(
                    "(n p) m -> p n m", p=128
                ),
            )

        wk_src = wk8_d if qk8 else wk_d
        wq_src = wq8_d if qk8 else wq_d
        dma_x(nc.sync, 0)
        nc.sync.dma_start(
            out=wk_sb[:, :, :], in_=wk_src[:, :].rearrange("(a p) n -> p a n", p=128)
        )
        nc.sync.dma_start(
            out=wq_sb[:, :, :], in_=wq_src[:, :].rearrange("(a p) n -> p a n", p=128)
        )
        nc.gpsimd.dma_start(
            out=wv_sb[:, :, :],
            in_=wv_d[:, :].rearrange("(a p) n -> p a n", p=128),
        )
        m0q(0)
        dma_x(nc.sync, 1)
        m0q(1)
        dma_x(nc.sync, 2)
        dma_x(nc.sync, 3)
        m0q(2)
        m0q(3)
        nc.gpsimd.dma_start(
            out=wo_sb[:, :, :],
            in_=wo_d[:, :].rearrange("(a p) n -> p a n", p=128),
        )

        qk_open = {}

        qk_psq = {}

        def emit_qk_piece(hp, w_sb, dst, qtc, piece):
            """Half-width (256 q-col) piece of a q/k projection chunk; the
            second piece drains the shared psum. Keeps PE lumps <= ~0.9us."""
            key = (id(dst), hp, qtc)
            if piece == 0:
                psq_new = auxps.tile([128, 512], F32, tag="aux")
                qk_psq[key] = psq_new
            psq = qk_psq[key]
            c0 = qtc * QB + piece * 256
            if qk8:
                # DoubleRow fp8: 2 contraction slabs per matmul, full 128 cols
                for pp in range(NP // 2):
                    nc.tensor.matmul(
                        psq[:, piece * 256 : (piece + 1) * 256],
                        lhsT=w_sb[:, 2 * pp : 2 * pp + 2, hp * 128 : (hp + 1) * 128],
                        rhs=x8[:, 2 * pp : 2 * pp + 2, c0 : c0 + 256],
                        start=(pp == 0),
                        stop=(pp == NP // 2 - 1),
                        perf_mode=DR,
                    )
            else:
                for p in range(NP):
                    nc.tensor.matmul(
                        psq[:, piece * 256 : (piece + 1) * 256],
                        lhsT=w_sb[:, p, hp * 128 : (hp + 1) * 128],
                        rhs=xts[:, p, c0 : c0 + 256],
                        start=(p == 0),
                        stop=(p == NP - 1),
                    )
            if piece == 1:
                nc.vector.tensor_copy(
                    dst[:, hp, qtc * QB : (qtc + 1) * QB], psq[:, :]
                )
                del qk_psq[key]

        def emit_qk_half(hp, w_sb, dst, pair, half):
            qtc = 2 * pair + half
            emit_qk_piece(hp, w_sb, dst, qtc, 0)
            emit_qk_piece(hp, w_sb, dst, qtc, 1)

        def emit_qk_pair(hp, w_sb, dst, pair):
            for half in range(2):
                emit_qk_half(hp, w_sb, dst, pair, half)

        def emit_v_proj(kt):
            ps_v = auxps.tile([128, 512], F32, tag="aux")
            for p in range(NP):
                nc.tensor.matmul(
                    ps_v[:, 0:INNER],
                    lhsT=xv[:, p, kt * 128 : (kt + 1) * 128],
                    rhs=wv_sb[:, p, :],
                    start=(p == 0),
                    stop=(p == NP - 1),
                )
            nc.vector.tensor_copy(
                v_sb[:, kt, :, 0:DH],
                ps_v[:, 0:INNER].rearrange("p (h d) -> p h d", h=HL),
            )

        # projection prefix: kT(hp0) keys 0:512 + qT(hp0) q 0:512; the rest
        # is interleaved into the block pipeline below
        emit_qk_half(0, wk_sb, kT, 0, 0)
        emit_qk_half(0, wq_sb, qT, 0, 0)

        anchor = persist.tile([128, 8], F32)
        if upto != "full":
            nc.vector.memset(anchor[:, :], 0.0)
        npool = ctx.enter_context(tc.tile_pool(name=f"np{rep}", bufs=8))
        natp = ctx.enter_context(tc.tile_pool(name=f"nat{rep}", bufs=8))
        epool = ctx.enter_context(tc.tile_pool(name=f"ep{rep}", bufs=2))
        opool = ctx.enter_context(tc.tile_pool(name=f"op{rep}", bufs=3))

        def make_pvt(qt, hp, e_t, qs_list=None):
            def emit():
                qss = list(qs_list if qs_list is not None else range(QB // 128))
                trp = None
                for qi, qs in enumerate(qss):
                    o_nat2 = natp.tile([128, 2, DH], BF16, tag="nat")
                    for hl in range(2):
                        head = 2 * hp + hl
                        pvt = pvtps.tile([128, 512], F32, tag="pvt")
                        for kt in range(NK):
                            nc.tensor.matmul(
                                pvt[:, 0 : DH + 1],
                                lhsT=e_t[:, hl, kt, qs * 128 : (qs + 1) * 128],
                                rhs=v_sb[:, kt, head, :],
                                start=(kt == 0),
                                stop=(kt == NK - 1),
                            )
                        if upto in ("pv",):
                            nc.vector.tensor_add(
                                anchor[:, :], anchor[:, :], pvt[:, 0:8]
                            )
                            continue
                        rc = npool.tile([128, 1], F32, tag="rc")
                        nc.vector.reciprocal_approx_fast(
                            out=rc[:, :], in_=pvt[:, DH : DH + 1]
                        )
                        nc.vector.tensor_scalar(
                            out=o_nat2[:, hl, :], in0=pvt[:, 0:DH],
                            scalar1=rc[:, :], scalar2=None, op0=MUL,
                        )
                    if upto in ("pv",):
                        continue
                    trp = pvtps.tile([128, 512], BF16, tag="pvt")
                    nc.tensor.transpose(
                        trp[0:128, 0:128],
                        o_nat2[:, :, :].rearrange("p a b -> p (a b)"),
                        ident[:, :],
                    )
                    qx = qt * (QB // 128) + qs
                    nc.vector.tensor_copy(
                        o_norm[:, hp, qx * 128 : (qx + 1) * 128],
                        trp[0:128, 0:128],
                    )

            return emit

        def make_proj(qt, sq_list=None):
            def emit():
                for sq in (sq_list if sq_list is not None else range(QB // 128)):
                    qx = qt * (QB // 128) + sq
                    o_sb = opool.tile([128, D], F32, tag="o")
                    for nh in range(D // 512):
                        fo = auxps.tile([128, 512], F32, tag="aux")
                        for hp2 in range(NHP):
                            nc.tensor.matmul(
                                fo[:, :],
                                lhsT=o_norm[:, hp2, qx * 128 : (qx + 1) * 128],
                                rhs=wo_sb[:, hp2, nh * 512 : (nh + 1) * 512],
                                start=(hp2 == 0),
                                stop=(hp2 == NHP - 1),
                            )
                        nc.vector.tensor_copy(
                            o_sb[:, nh * 512 : (nh + 1) * 512], fo[:, :]
                        )
                    nc.gpsimd.dma_start(
                        out=out_d[qx * 128 : (qx + 1) * 128, :], in_=o_sb[:, :]
                    )

            return emit

        def qkh(hp, w, d, pair, half):
            return lambda: emit_qk_half(hp, w, d, pair, half)

        def vg(k):
            return lambda: emit_v_proj(k)

        def qkp(hp, w, d, qtc, piece):
            return lambda: emit_qk_piece(hp, w, d, qtc, piece)

        # ordered by expected input-ready time (x chunks stream in serially);
        # a stalled PE instruction blocks everything behind it, so emission
        # order must track DMA arrival order
        FILL = {
            (0, 1): [qkp(0, wk_sb, kT, 1, 0)],
            (0, 2): [qkp(0, wk_sb, kT, 1, 1)],
            (0, 3): [vg(0)],
            (0, 4): [qkp(0, wk_sb, kT, 2, 0)],
            (0, 5): [qkp(0, wk_sb, kT, 2, 1)],
            (0, 6): [vg(1)],
            (0, 7): [qkp(0, wq_sb, qT, 1, 0)],
            (0, 8): [qkp(0, wq_sb, qT, 1, 1)],
            (0, 9): [vg(2)],
            (0, 10): [qkp(0, wk_sb, kT, 3, 0)],
            (0, 11): [qkp(0, wk_sb, kT, 3, 1)],
            (0, 12): [qkp(1, wk_sb, kT, 0, 0)],
            (0, 13): [qkp(1, wk_sb, kT, 0, 1)],
            (0, 14): [qkp(1, wq_sb, qT, 0, 0)],
            (0, 15): [qkp(1, wq_sb, qT, 0, 1)],
            (1, 1): [qkp(1, wk_sb, kT, 1, 0)],
            (1, 2): [qkp(1, wk_sb, kT, 1, 1)],
            (1, 3): [vg(3), vg(4)],
            (1, 4): [qkp(1, wk_sb, kT, 2, 0)],
            (1, 5): [qkp(1, wk_sb, kT, 2, 1)],
            (1, 6): [vg(5), vg(6)],
            (1, 7): [qkp(1, wk_sb, kT, 3, 0)],
            (1, 8): [qkp(1, wk_sb, kT, 3, 1)],
            (1, 9): [vg(7), vg(8)],
            (1, 10): [vg(9), vg(10)],
            (1, 11): [vg(11), vg(12)],
            (1, 12): [vg(13), vg(14)],
            (1, 13): [vg(15)],
            (2, 3): [qkp(0, wq_sb, qT, 2, 0)],
            (2, 4): [qkp(0, wq_sb, qT, 2, 1)],
            (2, 6): [qkp(1, wq_sb, qT, 1, 0)],
            (2, 7): [qkp(1, wq_sb, qT, 1, 1)],
            (3, 3): [qkp(0, wq_sb, qT, 3, 0)],
            (3, 4): [qkp(0, wq_sb, qT, 3, 1)],
            (3, 6): [qkp(1, wq_sb, qT, 2, 0)],
            (3, 7): [qkp(1, wq_sb, qT, 2, 1)],
            (4, 3): [qkp(1, wq_sb, qT, 3, 0)],
            (4, 4): [qkp(1, wq_sb, qT, 3, 1)],
        }
        PVT_POP = {(b, k) for b in range(2, 8) for k in (2, 4, 6, 8, 10, 12)} | {
            (1, 14), (1, 15)}
        PROJ_POP = {(b, k) for b in range(8) for k in (8, 12)}
        pending_pvt = []
        pending_proj = []
        blocks = [(qt, hp) for qt in range(NQ) for hp in range(NHP)]
        mb_cur = None
        for bi, (qt, hp) in enumerate(blocks):
            def emit_mask_dma(mqt):
                mb = mpool.tile([128, NK, QB], U16, tag="m")
                for mh in range(2):
                    meng_d = nc.sync if mh == 0 else nc.gpsimd
                    meng_d.dma_start(
                        out=mb[:, mh * 8 : (mh + 1) * 8, :],
                        in_=maskT_d[
                            mh * 1024 : (mh + 1) * 1024, mqt * QB : (mqt + 1) * QB
                        ].rearrange("(n p) m -> p n m", p=128),
                    )
                return mb

            if bi == 0:
                mb_cur = mb0
            elif hp == 0:
                mb_cur = mb_next
            e_t = epool.tile([128, 2, NK, QB], BF16, tag="e")
            HK = NK // 2
            for kt in range(NK):
                ps = simps.tile([128, 2, QB], F32, tag="sim")
                nc.tensor.matmul(
                    ps[:, 0, :],
                    lhsT=kT[0:64, hp, kt * 128 : (kt + 1) * 128],
                    rhs=qT[0:64, hp, qt * QB : (qt + 1) * QB],
                    start=True,
                    stop=True,
                    tile_position=(0, 0),
                )
                nc.tensor.matmul(
                    ps[:, 1, :],
                    lhsT=kT[64:128, hp, kt * 128 : (kt + 1) * 128],
                    rhs=qT[64:128, hp, qt * QB : (qt + 1) * QB],
                    start=True,
                    stop=True,
                    tile_position=(64, 0),
                )
                nc.scalar.activation(
                    e_t[:, :, kt, :], ps[:, :, :], AF.Exp, scale=scale
                )
                if upto == "sim":
                    continue
                if kt in (5, 9, 13):
                    # mask e &= mask (u16 0xFFFF/0x0000) in 4-kt chunks so the
                    # DVE never stalls psum drains for long
                    c0 = kt - 5
                    for hl in range(2):
                        ea = e_t[:, hl, c0 : c0 + 4, :].bitcast(U16)
                        nc.vector.tensor_tensor(
                            ea, ea, mb_cur[:, c0 : c0 + 4, :], AND
                        )
                for th in FILL.get((bi, kt), ()):
                    th()
                if hp == 1 and kt == 10 and qt + 1 < NQ:
                    mb_next = emit_mask_dma(qt + 1)
                if (bi, kt) in PVT_POP and pending_pvt:
                    pending_pvt.pop(0)()
                if (bi, kt) in PROJ_POP and pending_proj:
                    pending_proj.pop(0)()

            if upto != "sim":
                for hl in range(2):
                    ea = e_t[:, hl, 12:NK, :].bitcast(U16)
                    nc.vector.tensor_tensor(ea, ea, mb_cur[:, 12:NK, :], AND)
            if upto in ("sim", "mask"):
                nc.vector.tensor_add(
                    anchor[:, :], anchor[:, :], e_t[:, 0, NK - 1, 0:8]
                )
                continue
            pending_pvt.extend(
                make_pvt(qt, hp, e_t, [qs]) for qs in range(QB // 128)
            )
            if hp == NHP - 1 and upto not in ("pv", "norm"):
                pending_proj.extend(
                    make_proj(qt, [sq]) for sq in range(QB // 128)
                )
        while pending_pvt:
            pending_pvt.pop(0)()
        while pending_proj:
            pending_proj.pop(0)()
        if upto in ("sim", "mask", "pv"):
            nc.sync.dma_start(out=out_d[128:256, 0:8], in_=anchor[:, :])
        elif upto == "norm":
            sbp = persist.tile([128, 64], F32)
            nc.vector.tensor_copy(sbp[:, :], o_norm[0:128, 0, 0:64])
            nc.sync.dma_start(out=out_d[128:256, 0:64], in_=sbp[:, :])

    nc.compile()
    return nc


_NC_CACHE = {}
QK8 = False  # fp8 q/k projections: rel err ~0.06 > 2e-2 gate, keep bf16


def _get_nc():
    if "nc" not in _NC_CACHE:
        _NC_CACHE["nc"] = build_attention_nc(qk8=QK8)
    return _NC_CACHE["nc"]


def prep_in_maps(x, mask, Wq, Wk, Wv, Wo, qk8=None):
    """Per-core input dicts: core c -> batch c//4, head-group c%4."""
    if qk8 is None:
        qk8 = QK8
    BF = ml_dtypes.bfloat16
    F8 = ml_dtypes.float8_e4m3
    G, INNER = 4, 256
    ident = np.eye(128, dtype=BF)
    maskT_by_b, xT_by_b, xT8_by_b = {}, {}, {}
    in_maps = []
    for c in range(8):
        b, g = c // G, c % G
        if b not in maskT_by_b:
            maskT_by_b[b] = np.where(mask[b].T, np.uint16(0xFFFF), np.uint16(0))
            xT_by_b[b] = np.ascontiguousarray(
                np.asarray(x[b], dtype=np.float32).T
            ).astype(BF)
            if qk8:
                xT8_by_b[b] = xT_by_b[b].astype(F8)
        cols = slice(g * INNER, (g + 1) * INNER)
        Wqc = np.asarray(Wq[:, cols], dtype=np.float32)
        Wkc = np.asarray(Wk[:, cols], dtype=np.float32)
        m = {
            "maskT": maskT_by_b[b],
            "Wv": np.ascontiguousarray(np.asarray(Wv[:, cols], np.float32)).astype(BF),
            "Wo": np.ascontiguousarray(np.asarray(Wo[cols, :], np.float32)).astype(BF),
            "ident": ident,
        }
        if qk8:
            # 16x pre-scale keeps W in fp8's normal range; exp scale folds it out
            m["xT8"] = xT8_by_b[b]
            m["Wq8"] = np.ascontiguousarray(Wqc * 16.0).astype(F8)
            m["Wk8"] = np.ascontiguousarray(Wkc * 16.0).astype(F8)
        else:
            m["xT"] = xT_by_b[b]
            m["Wq"] = np.ascontiguousarray(Wqc).astype(BF)
            m["Wk"] = np.ascontiguousarray(Wkc).astype(BF)
        in_maps.append(m)
    return in_maps


def kernel(x, mask, Wq, Wk, Wv, Wo, bo):
    from concourse.bass_utils import run_bass_kernel_spmd

    x = np.asarray(x, dtype=np.float32)
    mask = np.asarray(mask)
    bo = np.asarray(bo, dtype=np.float32)
    B, S, D = x.shape
    G = 4  # head-groups per batch

    in_maps = prep_in_maps(x, mask, Wq, Wk, Wv, Wo)
    res = run_bass_kernel_spmd(_get_nc(), in_maps, core_ids=list(range(8)))
    outs = [r["out"] for r in res.results]
    full = np.empty((B, S, D), dtype=np.float32)
    for b in range(B):
        acc = outs[b * G].astype(np.float32, copy=True)
        for g in range(1, G):
            acc += outs[b * G + g]
        full[b] = acc + bo[None, :]
    return full

